# revision 6
# baseline (speedup 1.0000x reference)
"""Per-pixel dynamic 5x5 conv (KernelConv2d) + leaky-relu, data-parallel on 8 TRN2 cores.

Sharding: core i <- (n = i//2, h-half = i%2); each core computes out[n, :, h0:h0+128, :].
Per-core compute: out[c,h,w] = sum_{dy,dx} x[c, h+dy-2, w+dx-2] * k[c, dy*5+dx, h, w]
(replicate padding), then leaky_relu(0.2).

On-chip layout: partitions = 128 output rows. Each partition stores its own
5-row replicate-padded x window (per channel), duplicated at two byte
alignments (even/odd dx) so every tensor_tensor read is 4B-aligned and the
DVE 2x_1P fp16 packing mode applies. Kernels stream per-dy (5 taps at a
time) from DRAM in fully contiguous 2.6 MB DMAs. 25 fp16 multiplies +
24 fp16 adds per core on VectorE; leaky-relu fused into one
scalar_tensor_tensor (max(0.2*x, x)); output DMA'd contiguously.
"""

import os
from contextlib import ExitStack

import numpy as np

import concourse.bass as bass
import concourse.mybir as mybir
import concourse.tile as tile
from concourse.bass_utils import run_bass_kernel_spmd

N, C, H, W = 4, 8, 256, 256
K = 5
PAD = 2
NCORES = 8
HSH = H // 2            # 128 output rows per core
XW = 264                # stored row width (padded 260 -> 264 for alignment)
CD = mybir.dt.float16   # compute/storage dtype on chip
NEG = 0.2

_NC_CACHE = {}


KCW = K * C * W          # 10240 fp16 kernel elems per partition per dy
XCW = 2 * C * XW         # 4224 fp16 x-window elems per partition per dy
ROW = KCW + XCW          # 14464


def _build_nc():
    nc = bass.Bass("TRN2", target_bir_lowering=False, debug=False,
                   num_devices=NCORES)
    # xk[p, dy, ROW]: per-partition, per-dy packed row = 5 kernel taps
    # (dx,c,w) followed by the x window row (par,c,264cols). One DMA per dy.
    xk = nc.dram_tensor("xk", [HSH, K, ROW], CD, kind="ExternalInput").ap()
    out = nc.dram_tensor("out", [HSH, C, W], CD, kind="ExternalOutput").ap()

    # Raw bass (no TileContext): this walrus build allows only ONE sync-wait
    # per instruction, so all waits are emitted as standalone wait ops and
    # each DMA gets its own semaphore (a shared sem's 16 per-engine incs
    # interleave across in-flight DMAs and would fire early).
    with ExitStack() as ctx:
        xkt = [ctx.enter_context(nc.sbuf_tensor(f"xkt{i}", [HSH, ROW], CD)) for i in range(K)]
        gt = [ctx.enter_context(nc.sbuf_tensor(f"gt{i}", [HSH, C, W], CD)) for i in range(K)]
        pt = ctx.enter_context(nc.sbuf_tensor("pt", [HSH, C, W], CD))
        ot = ctx.enter_context(nc.sbuf_tensor("ot", [HSH, C, W], CD))
        s_k = [ctx.enter_context(nc.semaphore(f"sk{i}")) for i in range(K)]
        s_v = ctx.enter_context(nc.semaphore("sv"))
        s_o = ctx.enter_context(nc.semaphore("so"))
        block = ctx.enter_context(nc.Block())

        @block.sync
        def _(sync):
            for dy in range(K):
                sync.dma_start(xkt[dy][:], xk[:, dy]).then_inc(s_k[dy], 16)
            sync.wait_ge(s_v, 1)
            sync.dma_start(out[:], ot[:]).then_inc(s_o, 16)
            sync.wait_ge(s_o, 16)

        @block.vector
        def _(vector):
            for dy in range(K):
                vector.wait_ge(s_k[dy], 16)
                kv = xkt[dy][:, :KCW].rearrange("p (t c w) -> p t c w", t=K, c=C)
                xv = xkt[dy][:, KCW:].rearrange("p (q c w) -> p q c w", q=2, c=C)
                g = gt[dy]
                for dx in range(K):
                    par = dx & 1
                    off = dx - par
                    xs = xv[:, par, :, off:off + W]
                    ks = kv[:, dx]
                    if dx == 0:
                        vector.tensor_tensor(g[:], xs, ks, op=mybir.AluOpType.mult)
                    else:
                        vector.tensor_tensor(pt[:], xs, ks, op=mybir.AluOpType.mult)
                        vector.tensor_tensor(g[:], g[:], pt[:], op=mybir.AluOpType.add)
            g0, g1, g2, g3, g4 = (g[:] for g in gt)
            vector.tensor_tensor(g0, g0, g1, op=mybir.AluOpType.add)
            vector.tensor_tensor(g2, g2, g3, op=mybir.AluOpType.add)
            vector.tensor_tensor(g0, g0, g2, op=mybir.AluOpType.add)
            # leaky_relu(x, 0.2) == max(0.2*x, x); last add then fused lrelu
            vector.tensor_tensor(g0, g0, g4, op=mybir.AluOpType.add)
            vector.scalar_tensor_tensor(ot[:], g0, NEG, g0,
                                        op0=mybir.AluOpType.mult,
                                        op1=mybir.AluOpType.max).then_inc(s_v, 1)
    return nc


def get_nc():
    if "nc" not in _NC_CACHE:
        _NC_CACHE["nc"] = _build_nc()
    return _NC_CACHE["nc"]


def _prep_shards(x: np.ndarray, kernel: np.ndarray):
    """Host-side: pad, cast to fp16, build per-core DMA-friendly layouts."""
    f16 = np.float16
    # rows: replicate 2 each side; cols: 2 left, 9 right (264-wide even view +
    # one extra col so the odd-shifted view is in range; tail cols never read)
    xp = np.pad(x, ((0, 0), (0, 0), (PAD, PAD), (PAD, XW + 1 - W - PAD)),
                mode='edge').astype(f16)  # (N, C, 260, 265)
    kr = kernel.reshape(N, C, K * K, H, W)

    in_maps = []
    for core in range(NCORES):
        n, hb = divmod(core, 2)
        h0 = hb * HSH
        # sliding 5-row windows: win[c, p, r, w] = xp[n, c, h0+p+r, w]
        win = np.lib.stride_tricks.sliding_window_view(
            xp[n, :, h0:h0 + HSH + K - 1, :], K, axis=1)  # (C, 128, 265, 5)
        win = win.transpose(1, 3, 0, 2)  # (128, 5, C, 265)
        xd = np.stack([win[..., 0:XW], win[..., 1:XW + 1]], axis=2)
        xd = xd.reshape(HSH, K, XCW)
        kt = kr[n, :, :, h0:h0 + HSH, :].transpose(2, 1, 0, 3).astype(f16)
        kt = kt.reshape(HSH, K, KCW)
        xkb = np.ascontiguousarray(
            np.concatenate([kt, xd], axis=2))  # (128, 5, ROW)
        in_maps.append({"xk": xkb})
    return in_maps


def kernel(x: np.ndarray, kernel: np.ndarray) -> np.ndarray:
    nc = get_nc()
    in_maps = _prep_shards(np.asarray(x), np.asarray(kernel))
    trace = bool(int(os.environ.get("KC_TRACE", "0")))
    res = run_bass_kernel_spmd(nc, in_maps, core_ids=list(range(NCORES)),
                               trace=trace)
    _NC_CACHE["last_results"] = res
    out = np.empty((N, C, H, W), np.float32)
    for core in range(NCORES):
        n, hb = divmod(core, 2)
        h0 = hb * HSH
        o = res.results[core]["out"]  # (128, C, W) fp16
        out[n, :, h0:h0 + HSH, :] = o.transpose(1, 0, 2).astype(np.float32)
    return out
